# revision 15
# baseline (speedup 1.0000x reference)
"""AttentiveDensenet Trainium2 Bass kernel (v2).

Data-parallel over batch B=8 across 8 NeuronCores (1 image per core).

Key structure (vs v1): all conv/KQV weights are DMA'd in ONE batched
transfer per (layer, tensor) from host-packed layouts (the v1 per-tile
weight DMAs cost ~850us of sync-sequencer DIRECT2D issue time and
starved the PE into mid-pstate). The o pos-major -> channel-major
transpose is done on the PE (is_transpose matmuls against an identity)
instead of a DRAM xbar bounce. Attention arithmetic runs in bf16
(DVE 2x mode; validated 1.34e-2 rel err vs 2e-2 gate), split between
DVE and GpSimd, and is processed in two position-halves so conv1
chunk 0 and the o-transpose of half 0 overlap with attention of
half 1. BN AllGathers (one per channel-half) overlap conv1 of the
other half and conv2's ci=0 taps via a channel-half-split pipeline.

Per layer l (of 4):
  - K/Q/V 1x1 convs as bf16 matmuls, x-tiles stationary, position-major
    [pos, (head, dim)] output; bias via ones-row matmul into PSUM.
  - scores/softmax/top-k/weighted-sum on DVE (+GpSimd offload), bf16.
  - o transposed on PE into zero-padded conv input tiles.
  - conv3x3 #1 as 9 shifted 1x1 matmuls accumulated in PSUM.
  - BN stats (sum/sumsq) AllGathered across 8 cores (2KB); ob1 cancels
    in training-mode BN and is dropped. h1 = relu(A*y1+B) fused on Act.
  - conv3x3 #2 + residual x += gamma*(h2 + ob2) via scalar_tensor_tensor.
"""
import numpy as np
import ml_dtypes

import concourse.bacc as bacc
import concourse.mybir as mybir
import concourse.tile as tile
from concourse import bass_utils

L, C, B, H, W = 4, 256, 8, 32, 32
NH, KD = 8, 64
KH = NH * KD          # 512
HW = H * W            # 1024
P = 128
NC = 8                # cores
TOPK = 4
EPS = 1e-7
BN_EPS = 1e-5
PW = W + 2            # 34
PHW = PW * (H + 2)    # 1156
CHUNKS = [(0, 15), (15, 15), (30, 2)]

f32 = mybir.dt.float32
bf16 = mybir.dt.bfloat16
AX = mybir.AxisListType
OP = mybir.AluOpType
ACTF = mybir.ActivationFunctionType

_compiled = {}


def _build(ncores=NC, layers=L):
    nc = bacc.Bacc(None, target_bir_lowering=False, debug=False, num_devices=ncores)

    xin = nc.dram_tensor("xin", [C, HW], f32, kind="ExternalInput").ap()
    wkqvd = nc.dram_tensor("wkqvd", [L, P, 3 * 2 * KH], bf16, kind="ExternalInput").ap()
    bkd = nc.dram_tensor("bkd", [L, 1, 3 * KH], bf16, kind="ExternalInput").ap()
    w1d = nc.dram_tensor("w1d", [L, P, 72 * P], bf16, kind="ExternalInput").ap()
    w2d = nc.dram_tensor("w2d", [L, P, 36 * P], bf16, kind="ExternalInput").ap()
    cstd = nc.dram_tensor("cstd", [L, P, 8], f32, kind="ExternalInput").ap()
    identd = nc.dram_tensor("identd", [P, P], bf16, kind="ExternalInput").ap()
    out = nc.dram_tensor("out", [C, HW], f32, kind="ExternalOutput").ap()

    with tile.TileContext(nc) as tc, \
         nc.allow_low_precision(reason="bf16 attention validated vs reference"):
        with tc.tile_pool(name="main", bufs=1) as mp, \
             tc.tile_pool(name="prodp", bufs=4) as prodp, \
             tc.tile_pool(name="psp", bufs=6, space="PSUM") as psp, \
             tc.tile_pool(name="tpps", bufs=2, space="PSUM") as tpps, \
             tc.tile_pool(name="dramp", bufs=4, space="DRAM") as dramp:

            # ---- persistent tiles ----
            x = [mp.tile([P, HW], f32, name=f"x{i}") for i in range(2)]
            xb = [mp.tile([P, HW], bf16, name=f"xb{i}") for i in range(2)]
            qbt = mp.tile([P, 8 * KH], bf16, name="qbt")
            kbt = [mp.tile([P, 8 * KH], bf16, name=f"kbt{i}") for i in range(L)]
            vbt = [mp.tile([P, 8 * KH], bf16, name=f"vbt{i}") for i in range(L)]
            obf = mp.tile([P, 8 * KH], bf16, name="obf")
            S = mp.tile([P, 5 * 64], f32, name="S")
            attnb = mp.tile([P, 5 * 64], bf16, name="attnb")
            attn = mp.tile([P, 5 * 64], f32, name="attn")
            mx = mp.tile([P, 64], f32, name="mx")
            zs = mp.tile([P, 64], f32, name="zs")
            dmin = mp.tile([P, 64], f32, name="dmin")
            mxp = mp.tile([P, 64], f32, name="mxp")
            opad = [mp.tile([P, PHW + 2], bf16, name=f"opad{i}") for i in range(4)]
            y1 = [mp.tile([P, HW], f32, name=f"y1_{i}") for i in range(2)]
            sqs = [mp.tile([P, HW], f32, name=f"sqs{i}") for i in range(2)]
            h1p = [mp.tile([P, PHW + 2], bf16, name=f"h1p{i}") for i in range(2)]
            st = mp.tile([P, 4], f32, name="st")
            gst = mp.tile([P, 32], f32, name="gst")
            gsum = mp.tile([P, 4], f32, name="gsum")
            ones1 = mp.tile([1, P], bf16, name="ones1")
            ident = mp.tile([P, P], bf16, name="ident")
            # weight buffers (single-buffered; each reloads for layer l+1
            # right after its last layer-l consumer, hidden under compute)
            wkv = mp.tile([P, 3 * 2 * KH], bf16, name="wkv")
            bks = mp.tile([1, 3 * KH], bf16, name="bks")
            w1s = mp.tile([P, 72 * P], bf16, name="w1s")
            w2s = mp.tile([P, 36 * P], bf16, name="w2s")
            cst = mp.tile([P, 8], f32, name="cst")
            # BN scratch
            t1 = [mp.tile([P, 1], f32, name=f"t1_{i}") for i in range(2)]
            Ac = [mp.tile([P, 1], f32, name=f"Ac{i}") for i in range(2)]
            Bc = [mp.tile([P, 1], f32, name=f"Bc{i}") for i in range(2)]
            sq = [mp.tile([P, 1], f32, name=f"sq{i}") for i in range(2)]
            vart = [mp.tile([P, 1], f32, name=f"vart{i}") for i in range(2)]
            stdt = [mp.tile([P, 1], f32, name=f"stdt{i}") for i in range(2)]

            def copy_on(e, dst, src):
                if e is nc.scalar:
                    nc.scalar.copy(dst, src)
                else:
                    e.tensor_copy(dst, src)

            def load_kqv_weights(l):
                nc.sync.dma_start(wkv[:], wkqvd[l])
                nc.sync.dma_start(bks[:], bkd[l])

            # ---- init ----
            for i in range(2):
                nc.sync.dma_start(x[i][:], xin[i * P:(i + 1) * P, :])
                nc.scalar.copy(xb[i][:], x[i][:])
            nc.sync.dma_start(ident[:], identd)
            for i in range(4):
                nc.vector.memset(opad[i][:], 0)
            for i in range(2):
                nc.vector.memset(h1p[i][:], 0)
            nc.vector.memset(ones1[:], 1.0)
            load_kqv_weights(0)
            nc.sync.dma_start(w1s[:], w1d[0])
            nc.sync.dma_start(w2s[:], w2d[0])
            nc.sync.dma_start(cst[:], cstd[0])

            S3g = S[:].rearrange("p (t g) -> p g t", g=64)
            S3t = S[:].rearrange("p (t g) -> p t g", g=64)
            ab3g = attn[:].rearrange("p (t g) -> p g t", g=64)
            ab3t = attn[:].rearrange("p (t g) -> p t g", g=64)
            abb3t = attnb[:].rearrange("p (t g) -> p t g", g=64)

            for l in range(layers):
                R, T = l + 1, l + 2
                bng = [cst[:, 4 * co + 0:4 * co + 1] for co in range(2)]
                bnb = [cst[:, 4 * co + 1:4 * co + 2] for co in range(2)]
                gob2 = [cst[:, 4 * co + 2:4 * co + 3] for co in range(2)]
                gam = [cst[:, 4 * co + 3:4 * co + 4] for co in range(2)]

                # ---- KQV (both halves; PE streams while copies drain) ----
                cnt = 0
                for hb in range(2):
                    for c, dest in ((0, kbt[l]), (1, vbt[l]), (2, qbt)):
                        for pbh in range(4):
                            pb = hb * 4 + pbh
                            ps = psp.tile([P, KH], f32, name="ps", tag="ps")
                            nc.tensor.matmul(ps[:], ones1[:],
                                             bks[0:1, c * KH:(c + 1) * KH],
                                             start=True, stop=False)
                            nc.tensor.matmul(ps[:], xb[0][:, pb * P:(pb + 1) * P],
                                             wkv[:, (2 * c + 0) * KH:(2 * c + 1) * KH],
                                             start=False, stop=False)
                            nc.tensor.matmul(ps[:], xb[1][:, pb * P:(pb + 1) * P],
                                             wkv[:, (2 * c + 1) * KH:(2 * c + 2) * KH],
                                             start=False, stop=True)
                            copy_on(nc.scalar, dest[:, pb * KH:(pb + 1) * KH],
                                    ps[:])
                            cnt += 1
                if l + 1 < layers:
                    load_kqv_weights(l + 1)

                def conv1_part(parts):
                    for co, ck in parts:
                        i0, nr = CHUNKS[ck]
                        nw = PW * nr
                        ps = psp.tile([P, 512], f32, name="ps", tag="ps")
                        for tap in range(9):
                            ty, tx = divmod(tap, 3)
                            base = PW * (i0 + ty) + tx
                            for ci in range(4):
                                off = ((co * 9 + tap) * 4 + ci) * P
                                nc.tensor.matmul(ps[:, 0:nw], w1s[:, off:off + P],
                                                 opad[ci][:, base:base + nw],
                                                 start=(tap == 0 and ci == 0),
                                                 stop=(tap == 8 and ci == 3))
                        copy_on(nc.scalar,
                                y1[co][:, W * i0:W * (i0 + nr)].rearrange(
                                    "c (i j) -> c i j", j=W),
                                ps[:, 0:nw].rearrange("c (i j) -> c i j", j=PW)[:, :, 0:W])

                # ---- attention halves, pipelined with transpose + conv1 ----
                for hb in range(2):
                    CL = slice(hb * 2048, (hb + 1) * 2048)
                    GS = slice(hb * 32, hb * 32 + 32)
                    # scores
                    for t in range(R):
                        pr = prodp.tile([P, 2048], bf16, name="prod", tag="pr")
                        nc.vector.tensor_mul(pr[:], qbt[:, CL], kbt[t][:, CL])
                        st1 = prodp.tile([P, 2048], bf16, name="st1", tag="pr")
                        pr3 = pr[:].rearrange("p (g d) -> p g d", d=KD)
                        nc.vector.tensor_tensor(
                            st1[:, 0:1024].rearrange("p (g d) -> p g d", d=32),
                            pr3[:, :, 0:32], pr3[:, :, 32:64], OP.add)
                        nc.vector.tensor_reduce(
                            out=S3t[:, t, GS],
                            in_=st1[:, 0:1024].rearrange("p (g d) -> p g d", d=32),
                            axis=AX.X, op=OP.add)
                    nc.vector.memset(S3t[:, R, GS], 0)  # zero-key slot
                    # softmax over T slots
                    nc.vector.tensor_reduce(out=mx[:, GS], in_=S3g[:, GS, 0:T],
                                            axis=AX.X, op=OP.max)
                    nc.vector.tensor_tensor(
                        ab3g[:, GS, 0:T], S3g[:, GS, 0:T],
                        mx[:, GS].unsqueeze(2).broadcast_to([P, 32, T]), OP.subtract)
                    nc.scalar.activation(ab3t[:, 0:T, GS], ab3t[:, 0:T, GS], ACTF.Exp)
                    nc.vector.tensor_reduce(out=zs[:, GS], in_=ab3g[:, GS, 0:T],
                                            axis=AX.X, op=OP.add)
                    nc.vector.reciprocal(zs[:, GS], zs[:, GS])
                    nc.vector.tensor_tensor(
                        ab3g[:, GS, 0:T], ab3g[:, GS, 0:T],
                        zs[:, GS].unsqueeze(2).broadcast_to([P, 32, T]), OP.mult)
                    # sparse top-k (T=5 only): delta = 2nd-smallest = 4th-largest
                    if T > TOPK:
                        first = True
                        for i in range(T):
                            for j in range(i + 1, T):
                                dst = dmin if first else mxp
                                nc.vector.tensor_tensor(dst[:, GS], ab3t[:, i, GS],
                                                        ab3t[:, j, GS], OP.max)
                                if not first:
                                    nc.vector.tensor_tensor(dmin[:, GS], dmin[:, GS],
                                                            mxp[:, GS], OP.min)
                                first = False
                        nc.vector.tensor_scalar_add(dmin[:, GS], dmin[:, GS], EPS)
                        nc.vector.tensor_tensor(
                            ab3g[:, GS, 0:T], ab3g[:, GS, 0:T],
                            dmin[:, GS].unsqueeze(2).broadcast_to([P, 32, T]),
                            OP.subtract)
                        nc.vector.tensor_scalar_max(ab3g[:, GS, 0:T],
                                                    ab3g[:, GS, 0:T], 0.0)
                        nc.vector.tensor_reduce(out=zs[:, GS], in_=ab3g[:, GS, 0:T],
                                                axis=AX.X, op=OP.add)
                        nc.vector.tensor_scalar_add(zs[:, GS], zs[:, GS], EPS)
                        nc.vector.reciprocal(zs[:, GS], zs[:, GS])
                        nc.vector.tensor_tensor(
                            ab3g[:, GS, 0:T], ab3g[:, GS, 0:T],
                            zs[:, GS].unsqueeze(2).broadcast_to([P, 32, T]), OP.mult)
                    nc.vector.tensor_copy(abb3t[:, 0:T, GS], ab3t[:, 0:T, GS])
                    # weighted sum -> obf half. v/obf are d-major
                    # (col = pb*512 + d*8 + h) so the attn broadcast is
                    # packed in the last dim -> DVE 2x mode.
                    for t in range(R):
                        abb = abb3t[:, t, GS].rearrange(
                            "p (b h) -> p b h", h=8).unsqueeze(2).broadcast_to(
                            [P, 4, KD, 8])
                        vv = vbt[t][:, CL].rearrange(
                            "p (b d h) -> p b d h", d=KD, h=8)
                        if t == 0:
                            nc.vector.tensor_tensor(
                                obf[:, CL].rearrange("p (b d h) -> p b d h",
                                                     d=KD, h=8),
                                vv, abb, OP.mult)
                        else:
                            tm = prodp.tile([P, 2048], bf16, name="wtm", tag="pr")
                            nc.vector.tensor_tensor(
                                tm[:].rearrange("p (b d h) -> p b d h", d=KD, h=8),
                                vv, abb, OP.mult)
                            nc.vector.tensor_add(obf[:, CL], obf[:, CL], tm[:])
                    # PE transpose of this half into opad
                    for q in range(4):
                        tp = tpps.tile([P, 512], bf16, name="tp")
                        for pbh in range(4):
                            pb = hb * 4 + pbh
                            nc.tensor.matmul(
                                tp[:, pbh * P:(pbh + 1) * P],
                                obf[:, pb * KH + q * P: pb * KH + (q + 1) * P],
                                ident[:], is_transpose=True, skip_group_check=True)
                        opv = opad[q][:, 0:PHW].rearrange("c (i j) -> c i j", j=PW)
                        copy_on(nc.scalar,
                                opv[:, 1 + 16 * hb:17 + 16 * hb, 1:W + 1],
                                tp[:].rearrange("c (i j) -> c i j", j=W))
                    # conv1: chunk0 of both co overlap attention of half 1
                    if hb == 0:
                        conv1_part([(0, 0), (1, 0)])

                conv1_part([(0, 1), (0, 2)])

                # ---- stats + AllGather per channel-half, pipelined ----
                def stats(i):
                    nc.vector.tensor_reduce(out=st[:, 2 * i:2 * i + 1], in_=y1[i][:],
                                            axis=AX.X, op=OP.add)
                    nc.scalar.square(sqs[i][:], y1[i][:])
                    nc.vector.tensor_reduce(out=st[:, 2 * i + 1:2 * i + 2],
                                            in_=sqs[i][:], axis=AX.X, op=OP.add)
                    cci = dramp.tile([1, 2 * P], f32, name="cci")
                    cco = dramp.tile([ncores, 2 * P], f32, name="cco",
                                     addr_space="Shared")
                    nc.sync.dma_start(cci[0].rearrange("(p j) -> p j", j=2),
                                      st[:, 2 * i:2 * i + 2])
                    nc.gpsimd.collective_compute(
                        "AllGather", OP.bypass,
                        replica_groups=[list(range(ncores))],
                        ins=[cci.opt()], outs=[cco.opt()])
                    nc.sync.dma_start(
                        gst[:, i * 16:(i + 1) * 16].rearrange("p (j s) -> p j s",
                                                              s=ncores),
                        cco[:].rearrange("s (p j) -> p j s", j=2))

                stats(0)
                conv1_part([(1, 1), (1, 2)])
                stats(1)
                if l + 1 < layers:
                    nc.sync.dma_start(w1s[:], w1d[l + 1])
                for co in range(2):
                    nc.scalar.add(x[co][:], x[co][:], gob2[co])

                # ---- BN coef + h1 + conv2 (ci-split overlaps AG latency) ----
                NTOT = float(ncores * HW)

                def bn_h1(i):
                    nc.vector.tensor_reduce(
                        out=gsum[:, 2 * i:2 * i + 2],
                        in_=gst[:, i * 16:(i + 1) * 16].rearrange(
                            "p (j s) -> p j s", s=ncores),
                        axis=AX.X, op=OP.add)
                    nc.vector.tensor_scalar_mul(t1[i][:], gsum[:, 2 * i:2 * i + 1],
                                                1.0 / NTOT)
                    nc.vector.tensor_scalar_mul(vart[i][:],
                                                gsum[:, 2 * i + 1:2 * i + 2],
                                                1.0 / NTOT)
                    nc.vector.tensor_mul(sq[i][:], t1[i][:], t1[i][:])
                    nc.vector.tensor_sub(vart[i][:], vart[i][:], sq[i][:])
                    nc.vector.tensor_scalar_add(vart[i][:], vart[i][:], BN_EPS)
                    nc.scalar.activation(stdt[i][:], vart[i][:], ACTF.Sqrt)
                    nc.vector.reciprocal(stdt[i][:], stdt[i][:])
                    nc.vector.tensor_mul(Ac[i][:], bng[i], stdt[i][:])
                    nc.vector.tensor_mul(sq[i][:], t1[i][:], Ac[i][:])
                    nc.vector.tensor_sub(Bc[i][:], bnb[i], sq[i][:])
                    h1v = h1p[i][:, 0:PHW].rearrange("c (i j) -> c i j", j=PW)
                    nc.scalar.activation(
                        h1v[:, 1:H + 1, 1:W + 1],
                        y1[i][:].rearrange("c (i j) -> c i j", j=W),
                        ACTF.Relu, bias=Bc[i][:], scale=Ac[i][:])

                def conv2_taps(ps2, co, ci, start, stop):
                    for ck, (i0, nr) in enumerate(CHUNKS):
                        nw = PW * nr
                        for tap in range(9):
                            ty, tx = divmod(tap, 3)
                            base = PW * (i0 + ty) + tx
                            off = ((co * 9 + tap) * 2 + ci) * P
                            nc.tensor.matmul(
                                ps2[ck][:, 0:nw], w2s[:, off:off + P],
                                h1p[ci][:, base:base + nw],
                                start=(start and tap == 0),
                                stop=(stop and tap == 8))

                def resid(ps2, co):
                    for ck, (i0, nr) in enumerate(CHUNKS):
                        nw = PW * nr
                        xsl = x[co][:, W * i0:W * (i0 + nr)].rearrange(
                            "c (i j) -> c i j", j=W)
                        nc.vector.scalar_tensor_tensor(
                            out=xsl,
                            in0=ps2[ck][:, 0:nw].rearrange(
                                "c (i j) -> c i j", j=PW)[:, :, 0:W],
                            scalar=gam[co], in1=xsl, op0=OP.mult, op1=OP.add)
                    if l < layers - 1:
                        nc.scalar.copy(xb[co][:], x[co][:])
                    else:
                        nc.sync.dma_start(out[co * P:(co + 1) * P, :], x[co][:])

                # ci-split: both co groups' ci=0 taps hide AG(1) latency
                bn_h1(0)
                ps20 = [psp.tile([P, 512], f32, name="ps", tag="ps")
                        for _ in range(3)]
                ps21 = [psp.tile([P, 512], f32, name="ps", tag="ps")
                        for _ in range(3)]
                conv2_taps(ps20, 0, 0, True, False)
                conv2_taps(ps21, 1, 0, True, False)
                bn_h1(1)
                conv2_taps(ps20, 0, 1, False, True)
                resid(ps20, 0)
                conv2_taps(ps21, 1, 1, False, True)
                if l + 1 < layers:
                    nc.sync.dma_start(w2s[:], w2d[l + 1])
                resid(ps21, 1)
                if l + 1 < layers:
                    nc.sync.dma_start(cst[:], cstd[l + 1])

    nc.compile()
    return nc


def _host_prep(inputs):
    bf = ml_dtypes.bfloat16
    kw, kb = inputs["kw"], inputs["kb"]
    qw, qb = inputs["qw"], inputs["qb"]
    vw, vb = inputs["vw"], inputs["vb"]
    ow1, ow2 = inputs["ow1"], inputs["ow2"]
    ob2, gammas = inputs["ob2"], inputs["gammas"]

    def packkqv(w):  # [L, KH, C] -> [L, P, 2, KH]
        return w.reshape(L, KH, 2, P).transpose(0, 3, 2, 1)

    # v (and o) use d-major channel order: col d*8+h holds row h*64+d.
    # This makes the attention weighted-sum broadcast packed for DVE 2x.
    permdh = np.array([(c % 8) * 64 + c // 8 for c in range(KH)])
    d = {}
    wk3 = np.stack([packkqv(kw), packkqv(vw)[..., permdh],
                    packkqv(qw / 8.0)], axis=2)
    d["wkqvd"] = np.ascontiguousarray(wk3.reshape(L, P, 3 * 2 * KH)).astype(bf)
    bk3 = np.stack([kb, vb[:, permdh], qb / 8.0], axis=1)
    d["bkd"] = np.ascontiguousarray(bk3.reshape(L, 1, 3 * KH)).astype(bf)
    # conv1 input channels arrive via the PE transpose of d-major o:
    # opad[k] partition j holds channel c = (j%8)*64 + 16k + j//8.
    cinidx = np.array([[(j % 8) * 64 + 16 * k + j // 8 for j in range(P)]
                       for k in range(4)])
    a1 = ow1[:, :, cinidx]                      # [L, 256, 4, 128, 3, 3]
    a1 = a1.reshape(L, 2, P, 4, P, 3, 3).transpose(0, 4, 1, 5, 6, 3, 2)
    d["w1d"] = np.ascontiguousarray(a1.reshape(L, P, 72 * P)).astype(bf)
    a2 = ow2.reshape(L, 2, P, 2, P, 3, 3).transpose(0, 4, 1, 5, 6, 3, 2)
    d["w2d"] = np.ascontiguousarray(a2.reshape(L, P, 36 * P)).astype(bf)
    cstv = np.zeros((L, 2, P, 4), np.float32)
    cstv[..., 0] = inputs["bn_g"].reshape(L, 2, P)
    cstv[..., 1] = inputs["bn_b"].reshape(L, 2, P)
    cstv[..., 2] = (gammas[:, None] * ob2).reshape(L, 2, P)
    cstv[..., 3] = gammas[:, None, None]
    d["cstd"] = np.ascontiguousarray(
        cstv.transpose(0, 2, 1, 3).reshape(L, P, 8)).astype(np.float32)
    d["identd"] = np.eye(P, dtype=np.float32).astype(bf)
    return d


def kernel(**inputs):
    if "nc" not in _compiled:
        _compiled["nc"] = _build()
    nc = _compiled["nc"]
    shared = _host_prep(inputs)
    x = np.ascontiguousarray(inputs["x"].reshape(B, C, HW)).astype(np.float32)
    in_maps = []
    for c in range(NC):
        m = dict(shared)
        m["xin"] = x[c]
        in_maps.append(m)
    res = bass_utils.run_bass_kernel_spmd(nc, in_maps, core_ids=list(range(NC)))
    outs = np.stack([res.results[c]["out"] for c in range(NC)])
    return outs.reshape(B, C, H, W).astype(np.float32)


# revision 17
# speedup vs baseline: 1.0778x; 1.0778x over previous
"""AttentiveDensenet Trainium2 Bass kernel (v2).

Data-parallel over batch B=8 across 8 NeuronCores (1 image per core).

Key structure (vs v1): all conv/KQV weights are DMA'd in ONE batched
transfer per (layer, tensor) from host-packed layouts (the v1 per-tile
weight DMAs cost ~850us of sync-sequencer DIRECT2D issue time and
starved the PE into mid-pstate). The o pos-major -> channel-major
transpose is done on the PE (is_transpose matmuls against an identity)
instead of a DRAM xbar bounce. Attention arithmetic runs in bf16
(DVE 2x mode; validated 1.34e-2 rel err vs 2e-2 gate), split between
DVE and GpSimd, and is processed in two position-halves so conv1
chunk 0 and the o-transpose of half 0 overlap with attention of
half 1. BN AllGathers (one per channel-half) overlap conv1 of the
other half and conv2's ci=0 taps via a channel-half-split pipeline.

Per layer l (of 4):
  - K/Q/V 1x1 convs as bf16 matmuls, x-tiles stationary, position-major
    [pos, (head, dim)] output; bias via ones-row matmul into PSUM.
  - scores/softmax/top-k/weighted-sum on DVE (+GpSimd offload), bf16.
  - o transposed on PE into zero-padded conv input tiles.
  - conv3x3 #1 as 9 shifted 1x1 matmuls accumulated in PSUM.
  - BN stats (sum/sumsq) AllGathered across 8 cores (2KB); ob1 cancels
    in training-mode BN and is dropped. h1 = relu(A*y1+B) fused on Act.
  - conv3x3 #2 + residual x += gamma*(h2 + ob2) via scalar_tensor_tensor.
"""
import numpy as np
import ml_dtypes

import concourse.bacc as bacc
import concourse.mybir as mybir
import concourse.tile as tile
from concourse import bass_utils

L, C, B, H, W = 4, 256, 8, 32, 32
NH, KD = 8, 64
KH = NH * KD          # 512
HW = H * W            # 1024
P = 128
NC = 8                # cores
TOPK = 4
EPS = 1e-7
BN_EPS = 1e-5
PW = W + 2            # 34
PHW = PW * (H + 2)    # 1156
CHUNKS = [(0, 15), (15, 15), (30, 2)]

f32 = mybir.dt.float32
bf16 = mybir.dt.bfloat16
AX = mybir.AxisListType
OP = mybir.AluOpType
ACTF = mybir.ActivationFunctionType

_compiled = {}


def _build(ncores=NC, layers=L):
    nc = bacc.Bacc(None, target_bir_lowering=False, debug=False, num_devices=ncores)

    xin = nc.dram_tensor("xin", [C, HW], f32, kind="ExternalInput").ap()
    wkqvd = nc.dram_tensor("wkqvd", [L, P, 3 * 2 * KH], bf16, kind="ExternalInput").ap()
    bkd = nc.dram_tensor("bkd", [L, 1, 3 * KH], bf16, kind="ExternalInput").ap()
    w1d = nc.dram_tensor("w1d", [L, P, 72 * P], bf16, kind="ExternalInput").ap()
    w2d = nc.dram_tensor("w2d", [L, P, 36 * P], bf16, kind="ExternalInput").ap()
    cstd = nc.dram_tensor("cstd", [L, P, 8], f32, kind="ExternalInput").ap()
    identd = nc.dram_tensor("identd", [P, P], bf16, kind="ExternalInput").ap()
    out = nc.dram_tensor("out", [C, HW], f32, kind="ExternalOutput").ap()

    with tile.TileContext(nc) as tc, \
         nc.allow_low_precision(reason="bf16 attention validated vs reference"):
        with tc.tile_pool(name="main", bufs=1) as mp, \
             tc.tile_pool(name="prodp", bufs=4) as prodp, \
             tc.tile_pool(name="psp", bufs=6, space="PSUM") as psp, \
             tc.tile_pool(name="tpps", bufs=2, space="PSUM") as tpps, \
             tc.tile_pool(name="dramp", bufs=4, space="DRAM") as dramp:

            # ---- persistent tiles ----
            x = [mp.tile([P, HW], f32, name=f"x{i}") for i in range(2)]
            xb = [mp.tile([P, HW], bf16, name=f"xb{i}") for i in range(2)]
            qbt = mp.tile([P, 8 * KH], bf16, name="qbt")
            kbt = [mp.tile([P, 8 * KH], bf16, name=f"kbt{i}") for i in range(L)]
            vbt = [mp.tile([P, 8 * KH], bf16, name=f"vbt{i}") for i in range(L)]
            obf = mp.tile([P, 8 * KH], bf16, name="obf")
            S = mp.tile([P, 5 * 64], f32, name="S")
            attnb = mp.tile([P, 5 * 64], bf16, name="attnb")
            attn = mp.tile([P, 5 * 64], f32, name="attn")
            mx = mp.tile([P, 64], f32, name="mx")
            zs = mp.tile([P, 64], f32, name="zs")
            dmin = mp.tile([P, 64], f32, name="dmin")
            mxp = mp.tile([P, 64], f32, name="mxp")
            opad = [mp.tile([P, PHW + 2], bf16, name=f"opad{i}") for i in range(4)]
            y1 = [mp.tile([P, HW], f32, name=f"y1_{i}") for i in range(2)]
            sqs = [mp.tile([P, HW], f32, name=f"sqs{i}") for i in range(2)]
            h1p = [mp.tile([P, PHW + 2], bf16, name=f"h1p{i}") for i in range(2)]
            st = mp.tile([P, 4], f32, name="st")
            gst = mp.tile([P, 32], f32, name="gst")
            gsum = mp.tile([P, 4], f32, name="gsum")
            ones1 = mp.tile([1, P], bf16, name="ones1")
            ident = mp.tile([P, P], bf16, name="ident")
            # weight buffers (single-buffered; each reloads for layer l+1
            # right after its last layer-l consumer, hidden under compute)
            wkv = mp.tile([P, 3 * 2 * KH], bf16, name="wkv")
            bks = mp.tile([1, 3 * KH], bf16, name="bks")
            w1s = mp.tile([P, 72 * P], bf16, name="w1s")
            w2s = mp.tile([P, 36 * P], bf16, name="w2s")
            cst = mp.tile([P, 8], f32, name="cst")
            # BN scratch
            t1 = [mp.tile([P, 1], f32, name=f"t1_{i}") for i in range(2)]
            Ac = [mp.tile([P, 1], f32, name=f"Ac{i}") for i in range(2)]
            Bc = [mp.tile([P, 1], f32, name=f"Bc{i}") for i in range(2)]
            sq = [mp.tile([P, 1], f32, name=f"sq{i}") for i in range(2)]
            vart = [mp.tile([P, 1], f32, name=f"vart{i}") for i in range(2)]
            stdt = [mp.tile([P, 1], f32, name=f"stdt{i}") for i in range(2)]

            def copy_on(e, dst, src):
                if e is nc.scalar:
                    nc.scalar.copy(dst, src)
                else:
                    e.tensor_copy(dst, src)

            def load_kqv_weights(l):
                nc.sync.dma_start(wkv[:], wkqvd[l])
                nc.sync.dma_start(bks[:], bkd[l])

            # ---- init ----
            for i in range(2):
                nc.sync.dma_start(x[i][:], xin[i * P:(i + 1) * P, :])
                nc.scalar.copy(xb[i][:], x[i][:])
            nc.sync.dma_start(ident[:], identd)
            for i in range(4):
                nc.vector.memset(opad[i][:], 0)
            for i in range(2):
                nc.vector.memset(h1p[i][:], 0)
            nc.vector.memset(ones1[:], 1.0)
            load_kqv_weights(0)
            nc.sync.dma_start(w1s[:], w1d[0])
            nc.sync.dma_start(w2s[:], w2d[0])
            nc.sync.dma_start(cst[:], cstd[0])

            S3g = S[:].rearrange("p (t g) -> p g t", g=64)
            S3t = S[:].rearrange("p (t g) -> p t g", g=64)
            ab3g = attn[:].rearrange("p (t g) -> p g t", g=64)
            ab3t = attn[:].rearrange("p (t g) -> p t g", g=64)
            abb3t = attnb[:].rearrange("p (t g) -> p t g", g=64)

            for l in range(layers):
                R, T = l + 1, l + 2
                bng = [cst[:, 4 * co + 0:4 * co + 1] for co in range(2)]
                bnb = [cst[:, 4 * co + 1:4 * co + 2] for co in range(2)]
                gob2 = [cst[:, 4 * co + 2:4 * co + 3] for co in range(2)]
                gam = [cst[:, 4 * co + 3:4 * co + 4] for co in range(2)]

                # ---- KQV (both halves; PE streams while copies drain) ----
                cnt = 0
                for hb in range(2):
                    for c, dest in ((0, kbt[l]), (1, vbt[l]), (2, qbt)):
                        for pbh in range(4):
                            pb = hb * 4 + pbh
                            ps = psp.tile([P, KH], f32, name="ps", tag="ps")
                            nc.tensor.matmul(ps[:], ones1[:],
                                             bks[0:1, c * KH:(c + 1) * KH],
                                             start=True, stop=False)
                            nc.tensor.matmul(ps[:], xb[0][:, pb * P:(pb + 1) * P],
                                             wkv[:, (2 * c + 0) * KH:(2 * c + 1) * KH],
                                             start=False, stop=False)
                            nc.tensor.matmul(ps[:], xb[1][:, pb * P:(pb + 1) * P],
                                             wkv[:, (2 * c + 1) * KH:(2 * c + 2) * KH],
                                             start=False, stop=True)
                            e = (nc.scalar, nc.vector)[cnt % 2] \
                                if hb == 0 else nc.scalar
                            copy_on(e, dest[:, pb * KH:(pb + 1) * KH], ps[:])
                            cnt += 1
                if l + 1 < layers:
                    load_kqv_weights(l + 1)

                def conv1_part(parts):
                    for co, ck in parts:
                        i0, nr = CHUNKS[ck]
                        nw = PW * nr
                        ps = psp.tile([P, 512], f32, name="ps", tag="ps")
                        for tap in range(9):
                            ty, tx = divmod(tap, 3)
                            base = PW * (i0 + ty) + tx
                            for ci in range(4):
                                off = ((co * 9 + tap) * 4 + ci) * P
                                nc.tensor.matmul(ps[:, 0:nw], w1s[:, off:off + P],
                                                 opad[ci][:, base:base + nw],
                                                 start=(tap == 0 and ci == 0),
                                                 stop=(tap == 8 and ci == 3))
                        copy_on(nc.vector if ck % 2 else nc.scalar,
                                y1[co][:, W * i0:W * (i0 + nr)].rearrange(
                                    "c (i j) -> c i j", j=W),
                                ps[:, 0:nw].rearrange("c (i j) -> c i j", j=PW)[:, :, 0:W])

                # ---- attention halves, pipelined with transpose + conv1 ----
                for hb in range(2):
                    CL = slice(hb * 2048, (hb + 1) * 2048)
                    GS = slice(hb * 32, hb * 32 + 32)
                    # scores
                    for t in range(R):
                        pr = prodp.tile([P, 2048], bf16, name="prod", tag="pr")
                        nc.vector.tensor_mul(pr[:], qbt[:, CL], kbt[t][:, CL])
                        st1 = prodp.tile([P, 2048], bf16, name="st1", tag="pr")
                        pr3 = pr[:].rearrange("p (g d) -> p g d", d=KD)
                        nc.vector.tensor_tensor(
                            st1[:, 0:1024].rearrange("p (g d) -> p g d", d=32),
                            pr3[:, :, 0:32], pr3[:, :, 32:64], OP.add)
                        nc.vector.tensor_reduce(
                            out=S3t[:, t, GS],
                            in_=st1[:, 0:1024].rearrange("p (g d) -> p g d", d=32),
                            axis=AX.X, op=OP.add)
                    nc.vector.memset(S3t[:, R, GS], 0)  # zero-key slot
                    # softmax over T slots
                    nc.vector.tensor_reduce(out=mx[:, GS], in_=S3g[:, GS, 0:T],
                                            axis=AX.X, op=OP.max)
                    nc.vector.tensor_tensor(
                        ab3g[:, GS, 0:T], S3g[:, GS, 0:T],
                        mx[:, GS].unsqueeze(2).broadcast_to([P, 32, T]), OP.subtract)
                    nc.scalar.activation(ab3t[:, 0:T, GS], ab3t[:, 0:T, GS], ACTF.Exp)
                    nc.vector.tensor_reduce(out=zs[:, GS], in_=ab3g[:, GS, 0:T],
                                            axis=AX.X, op=OP.add)
                    nc.vector.reciprocal(zs[:, GS], zs[:, GS])
                    nc.vector.tensor_tensor(
                        ab3g[:, GS, 0:T], ab3g[:, GS, 0:T],
                        zs[:, GS].unsqueeze(2).broadcast_to([P, 32, T]), OP.mult)
                    # sparse top-k (T=5 only): delta = 2nd-smallest = 4th-largest
                    if T > TOPK:
                        first = True
                        for i in range(T):
                            for j in range(i + 1, T):
                                dst = dmin if first else mxp
                                nc.vector.tensor_tensor(dst[:, GS], ab3t[:, i, GS],
                                                        ab3t[:, j, GS], OP.max)
                                if not first:
                                    nc.vector.tensor_tensor(dmin[:, GS], dmin[:, GS],
                                                            mxp[:, GS], OP.min)
                                first = False
                        nc.vector.tensor_scalar_add(dmin[:, GS], dmin[:, GS], EPS)
                        nc.vector.tensor_tensor(
                            ab3g[:, GS, 0:T], ab3g[:, GS, 0:T],
                            dmin[:, GS].unsqueeze(2).broadcast_to([P, 32, T]),
                            OP.subtract)
                        nc.vector.tensor_scalar_max(ab3g[:, GS, 0:T],
                                                    ab3g[:, GS, 0:T], 0.0)
                        nc.vector.tensor_reduce(out=zs[:, GS], in_=ab3g[:, GS, 0:T],
                                                axis=AX.X, op=OP.add)
                        nc.vector.tensor_scalar_add(zs[:, GS], zs[:, GS], EPS)
                        nc.vector.reciprocal(zs[:, GS], zs[:, GS])
                        nc.vector.tensor_tensor(
                            ab3g[:, GS, 0:T], ab3g[:, GS, 0:T],
                            zs[:, GS].unsqueeze(2).broadcast_to([P, 32, T]), OP.mult)
                    nc.vector.tensor_copy(abb3t[:, 0:T, GS], ab3t[:, 0:T, GS])
                    # weighted sum -> obf half. v/obf are d-major
                    # (col = pb*512 + d*8 + h) so the attn broadcast is
                    # packed in the last dim -> DVE 2x mode.
                    for t in range(R):
                        abb = abb3t[:, t, GS].rearrange(
                            "p (b h) -> p b h", h=8).unsqueeze(2).broadcast_to(
                            [P, 4, KD, 8])
                        vv = vbt[t][:, CL].rearrange(
                            "p (b d h) -> p b d h", d=KD, h=8)
                        if t == 0:
                            nc.vector.tensor_tensor(
                                obf[:, CL].rearrange("p (b d h) -> p b d h",
                                                     d=KD, h=8),
                                vv, abb, OP.mult)
                        else:
                            tm = prodp.tile([P, 2048], bf16, name="wtm", tag="pr")
                            nc.vector.tensor_tensor(
                                tm[:].rearrange("p (b d h) -> p b d h", d=KD, h=8),
                                vv, abb, OP.mult)
                            nc.vector.tensor_add(obf[:, CL], obf[:, CL], tm[:])
                    # PE transpose of this half into opad
                    for q in range(4):
                        tp = tpps.tile([P, 512], bf16, name="tp")
                        for pbh in range(4):
                            pb = hb * 4 + pbh
                            nc.tensor.matmul(
                                tp[:, pbh * P:(pbh + 1) * P],
                                obf[:, pb * KH + q * P: pb * KH + (q + 1) * P],
                                ident[:], is_transpose=True, skip_group_check=True)
                        opv = opad[q][:, 0:PHW].rearrange("c (i j) -> c i j", j=PW)
                        copy_on(nc.scalar,
                                opv[:, 1 + 16 * hb:17 + 16 * hb, 1:W + 1],
                                tp[:].rearrange("c (i j) -> c i j", j=W))
                    # conv1: chunk0 of both co overlap attention of half 1
                    if hb == 0:
                        conv1_part([(0, 0), (1, 0)])

                conv1_part([(0, 1), (0, 2)])

                # ---- stats + AllGather per channel-half, pipelined ----
                def stats(i):
                    nc.vector.tensor_reduce(out=st[:, 2 * i:2 * i + 1], in_=y1[i][:],
                                            axis=AX.X, op=OP.add)
                    nc.scalar.square(sqs[i][:], y1[i][:])
                    nc.vector.tensor_reduce(out=st[:, 2 * i + 1:2 * i + 2],
                                            in_=sqs[i][:], axis=AX.X, op=OP.add)
                    cci = dramp.tile([1, 2 * P], f32, name="cci")
                    cco = dramp.tile([ncores, 2 * P], f32, name="cco",
                                     addr_space="Shared")
                    nc.sync.dma_start(cci[0].rearrange("(p j) -> p j", j=2),
                                      st[:, 2 * i:2 * i + 2])
                    nc.gpsimd.collective_compute(
                        "AllGather", OP.bypass,
                        replica_groups=[list(range(ncores))],
                        ins=[cci.opt()], outs=[cco.opt()])
                    nc.sync.dma_start(
                        gst[:, i * 16:(i + 1) * 16].rearrange("p (j s) -> p j s",
                                                              s=ncores),
                        cco[:].rearrange("s (p j) -> p j s", j=2))

                stats(0)
                conv1_part([(1, 1), (1, 2)])
                stats(1)
                if l + 1 < layers:
                    nc.sync.dma_start(w1s[:], w1d[l + 1])
                for co in range(2):
                    nc.scalar.add(x[co][:], x[co][:], gob2[co])

                # ---- BN coef + h1 + conv2 (ci-split overlaps AG latency) ----
                NTOT = float(ncores * HW)

                def bn_h1(i):
                    nc.vector.tensor_reduce(
                        out=gsum[:, 2 * i:2 * i + 2],
                        in_=gst[:, i * 16:(i + 1) * 16].rearrange(
                            "p (j s) -> p j s", s=ncores),
                        axis=AX.X, op=OP.add)
                    nc.vector.tensor_scalar_mul(t1[i][:], gsum[:, 2 * i:2 * i + 1],
                                                1.0 / NTOT)
                    nc.vector.tensor_scalar_mul(vart[i][:],
                                                gsum[:, 2 * i + 1:2 * i + 2],
                                                1.0 / NTOT)
                    nc.vector.tensor_mul(sq[i][:], t1[i][:], t1[i][:])
                    nc.vector.tensor_sub(vart[i][:], vart[i][:], sq[i][:])
                    nc.vector.tensor_scalar_add(vart[i][:], vart[i][:], BN_EPS)
                    nc.scalar.activation(stdt[i][:], vart[i][:], ACTF.Sqrt)
                    nc.vector.reciprocal(stdt[i][:], stdt[i][:])
                    nc.vector.tensor_mul(Ac[i][:], bng[i], stdt[i][:])
                    nc.vector.tensor_mul(sq[i][:], t1[i][:], Ac[i][:])
                    nc.vector.tensor_sub(Bc[i][:], bnb[i], sq[i][:])
                    h1v = h1p[i][:, 0:PHW].rearrange("c (i j) -> c i j", j=PW)
                    nc.scalar.activation(
                        h1v[:, 1:H + 1, 1:W + 1],
                        y1[i][:].rearrange("c (i j) -> c i j", j=W),
                        ACTF.Relu, bias=Bc[i][:], scale=Ac[i][:])

                def conv2_taps(ps2, co, ci, start, stop):
                    for ck, (i0, nr) in enumerate(CHUNKS):
                        nw = PW * nr
                        for tap in range(9):
                            ty, tx = divmod(tap, 3)
                            base = PW * (i0 + ty) + tx
                            off = ((co * 9 + tap) * 2 + ci) * P
                            nc.tensor.matmul(
                                ps2[ck][:, 0:nw], w2s[:, off:off + P],
                                h1p[ci][:, base:base + nw],
                                start=(start and tap == 0),
                                stop=(stop and tap == 8))

                def resid(ps2, co):
                    for ck, (i0, nr) in enumerate(CHUNKS):
                        nw = PW * nr
                        xsl = x[co][:, W * i0:W * (i0 + nr)].rearrange(
                            "c (i j) -> c i j", j=W)
                        nc.vector.scalar_tensor_tensor(
                            out=xsl,
                            in0=ps2[ck][:, 0:nw].rearrange(
                                "c (i j) -> c i j", j=PW)[:, :, 0:W],
                            scalar=gam[co], in1=xsl, op0=OP.mult, op1=OP.add)
                    if l < layers - 1:
                        nc.scalar.copy(xb[co][:], x[co][:])
                    else:
                        nc.sync.dma_start(out[co * P:(co + 1) * P, :], x[co][:])

                # ci-split: both co groups' ci=0 taps hide AG(1) latency
                bn_h1(0)
                ps20 = [psp.tile([P, 512], f32, name="ps", tag="ps")
                        for _ in range(3)]
                ps21 = [psp.tile([P, 512], f32, name="ps", tag="ps")
                        for _ in range(3)]
                conv2_taps(ps20, 0, 0, True, False)
                conv2_taps(ps21, 1, 0, True, False)
                bn_h1(1)
                conv2_taps(ps20, 0, 1, False, True)
                resid(ps20, 0)
                conv2_taps(ps21, 1, 1, False, True)
                if l + 1 < layers:
                    nc.sync.dma_start(w2s[:], w2d[l + 1])
                resid(ps21, 1)
                if l + 1 < layers:
                    nc.sync.dma_start(cst[:], cstd[l + 1])

    nc.compile()
    return nc


def _host_prep(inputs):
    bf = ml_dtypes.bfloat16
    kw, kb = inputs["kw"], inputs["kb"]
    qw, qb = inputs["qw"], inputs["qb"]
    vw, vb = inputs["vw"], inputs["vb"]
    ow1, ow2 = inputs["ow1"], inputs["ow2"]
    ob2, gammas = inputs["ob2"], inputs["gammas"]

    def packkqv(w):  # [L, KH, C] -> [L, P, 2, KH]
        return w.reshape(L, KH, 2, P).transpose(0, 3, 2, 1)

    # v (and o) use d-major channel order: col d*8+h holds row h*64+d.
    # This makes the attention weighted-sum broadcast packed for DVE 2x.
    permdh = np.array([(c % 8) * 64 + c // 8 for c in range(KH)])
    d = {}
    wk3 = np.stack([packkqv(kw), packkqv(vw)[..., permdh],
                    packkqv(qw / 8.0)], axis=2)
    d["wkqvd"] = np.ascontiguousarray(wk3.reshape(L, P, 3 * 2 * KH)).astype(bf)
    bk3 = np.stack([kb, vb[:, permdh], qb / 8.0], axis=1)
    d["bkd"] = np.ascontiguousarray(bk3.reshape(L, 1, 3 * KH)).astype(bf)
    # conv1 input channels arrive via the PE transpose of d-major o:
    # opad[k] partition j holds channel c = (j%8)*64 + 16k + j//8.
    cinidx = np.array([[(j % 8) * 64 + 16 * k + j // 8 for j in range(P)]
                       for k in range(4)])
    a1 = ow1[:, :, cinidx]                      # [L, 256, 4, 128, 3, 3]
    a1 = a1.reshape(L, 2, P, 4, P, 3, 3).transpose(0, 4, 1, 5, 6, 3, 2)
    d["w1d"] = np.ascontiguousarray(a1.reshape(L, P, 72 * P)).astype(bf)
    a2 = ow2.reshape(L, 2, P, 2, P, 3, 3).transpose(0, 4, 1, 5, 6, 3, 2)
    d["w2d"] = np.ascontiguousarray(a2.reshape(L, P, 36 * P)).astype(bf)
    cstv = np.zeros((L, 2, P, 4), np.float32)
    cstv[..., 0] = inputs["bn_g"].reshape(L, 2, P)
    cstv[..., 1] = inputs["bn_b"].reshape(L, 2, P)
    cstv[..., 2] = (gammas[:, None] * ob2).reshape(L, 2, P)
    cstv[..., 3] = gammas[:, None, None]
    d["cstd"] = np.ascontiguousarray(
        cstv.transpose(0, 2, 1, 3).reshape(L, P, 8)).astype(np.float32)
    d["identd"] = np.eye(P, dtype=np.float32).astype(bf)
    return d


def kernel(**inputs):
    if "nc" not in _compiled:
        _compiled["nc"] = _build()
    nc = _compiled["nc"]
    shared = _host_prep(inputs)
    x = np.ascontiguousarray(inputs["x"].reshape(B, C, HW)).astype(np.float32)
    in_maps = []
    for c in range(NC):
        m = dict(shared)
        m["xin"] = x[c]
        in_maps.append(m)
    res = bass_utils.run_bass_kernel_spmd(nc, in_maps, core_ids=list(range(NC)))
    outs = np.stack([res.results[c]["out"] for c in range(NC)])
    return outs.reshape(B, C, H, W).astype(np.float32)


# revision 18
# speedup vs baseline: 1.0846x; 1.0064x over previous
"""AttentiveDensenet Trainium2 Bass kernel (v2).

Data-parallel over batch B=8 across 8 NeuronCores (1 image per core).

Key structure (vs v1): all conv/KQV weights are DMA'd in ONE batched
transfer per (layer, tensor) from host-packed layouts (the v1 per-tile
weight DMAs cost ~850us of sync-sequencer DIRECT2D issue time and
starved the PE into mid-pstate). The o pos-major -> channel-major
transpose is done on the PE (is_transpose matmuls against an identity)
instead of a DRAM xbar bounce. Attention arithmetic runs in bf16
(DVE 2x mode; validated 1.34e-2 rel err vs 2e-2 gate), split between
DVE and GpSimd, and is processed in two position-halves so conv1
chunk 0 and the o-transpose of half 0 overlap with attention of
half 1. BN AllGathers (one per channel-half) overlap conv1 of the
other half and conv2's ci=0 taps via a channel-half-split pipeline.

Per layer l (of 4):
  - K/Q/V 1x1 convs as bf16 matmuls, x-tiles stationary, position-major
    [pos, (head, dim)] output; bias via ones-row matmul into PSUM.
  - scores/softmax/top-k/weighted-sum on DVE (+GpSimd offload), bf16.
  - o transposed on PE into zero-padded conv input tiles.
  - conv3x3 #1 as 9 shifted 1x1 matmuls accumulated in PSUM.
  - BN stats (sum/sumsq) AllGathered across 8 cores (2KB); ob1 cancels
    in training-mode BN and is dropped. h1 = relu(A*y1+B) fused on Act.
  - conv3x3 #2 + residual x += gamma*(h2 + ob2) via scalar_tensor_tensor.
"""
import numpy as np
import ml_dtypes

import concourse.bacc as bacc
import concourse.mybir as mybir
import concourse.tile as tile
from concourse import bass_utils

L, C, B, H, W = 4, 256, 8, 32, 32
NH, KD = 8, 64
KH = NH * KD          # 512
HW = H * W            # 1024
P = 128
NC = 8                # cores
TOPK = 4
EPS = 1e-7
BN_EPS = 1e-5
PW = W + 2            # 34
PHW = PW * (H + 2)    # 1156
CHUNKS = [(0, 15), (15, 15), (30, 2)]

f32 = mybir.dt.float32
bf16 = mybir.dt.bfloat16
AX = mybir.AxisListType
OP = mybir.AluOpType
ACTF = mybir.ActivationFunctionType

_compiled = {}


def _build(ncores=NC, layers=L):
    nc = bacc.Bacc(None, target_bir_lowering=False, debug=False, num_devices=ncores)

    xin = nc.dram_tensor("xin", [C, HW], f32, kind="ExternalInput").ap()
    wkqvd = nc.dram_tensor("wkqvd", [L, P, 3 * 2 * KH], bf16, kind="ExternalInput").ap()
    bkd = nc.dram_tensor("bkd", [L, 1, 3 * KH], bf16, kind="ExternalInput").ap()
    bkfd = nc.dram_tensor("bkfd", [L, P, 3 * KH], f32, kind="ExternalInput").ap()
    w1d = nc.dram_tensor("w1d", [L, P, 72 * P], bf16, kind="ExternalInput").ap()
    w2d = nc.dram_tensor("w2d", [L, P, 36 * P], bf16, kind="ExternalInput").ap()
    cstd = nc.dram_tensor("cstd", [L, P, 8], f32, kind="ExternalInput").ap()
    identd = nc.dram_tensor("identd", [P, P], bf16, kind="ExternalInput").ap()
    out = nc.dram_tensor("out", [C, HW], f32, kind="ExternalOutput").ap()

    with tile.TileContext(nc) as tc, \
         nc.allow_low_precision(reason="bf16 attention validated vs reference"):
        with tc.tile_pool(name="main", bufs=1) as mp, \
             tc.tile_pool(name="prodp", bufs=4) as prodp, \
             tc.tile_pool(name="psp", bufs=6, space="PSUM") as psp, \
             tc.tile_pool(name="tpps", bufs=2, space="PSUM") as tpps, \
             tc.tile_pool(name="dramp", bufs=4, space="DRAM") as dramp:

            # ---- persistent tiles ----
            x = [mp.tile([P, HW], f32, name=f"x{i}") for i in range(2)]
            xb = [mp.tile([P, HW], bf16, name=f"xb{i}") for i in range(2)]
            qbt = mp.tile([P, 8 * KH], bf16, name="qbt")
            kbt = [mp.tile([P, 8 * KH], bf16, name=f"kbt{i}") for i in range(L)]
            vbt = [mp.tile([P, 8 * KH], bf16, name=f"vbt{i}") for i in range(L)]
            obf = mp.tile([P, 8 * KH], bf16, name="obf")
            S = mp.tile([P, 5 * 64], f32, name="S")
            attnb = mp.tile([P, 5 * 64], bf16, name="attnb")
            attn = mp.tile([P, 5 * 64], f32, name="attn")
            mx = mp.tile([P, 64], f32, name="mx")
            zs = mp.tile([P, 64], f32, name="zs")
            dmin = mp.tile([P, 64], f32, name="dmin")
            mxp = mp.tile([P, 64], f32, name="mxp")
            opad = [mp.tile([P, PHW + 2], bf16, name=f"opad{i}") for i in range(4)]
            y1 = [mp.tile([P, HW], f32, name=f"y1_{i}") for i in range(2)]
            sqs = [mp.tile([P, HW], f32, name=f"sqs{i}") for i in range(2)]
            h1p = [mp.tile([P, PHW + 2], bf16, name=f"h1p{i}") for i in range(2)]
            st = mp.tile([P, 4], f32, name="st")
            gst = mp.tile([P, 32], f32, name="gst")
            gsum = mp.tile([P, 4], f32, name="gsum")
            ones1 = mp.tile([1, P], bf16, name="ones1")
            ident = mp.tile([P, P], bf16, name="ident")
            # weight buffers (single-buffered; each reloads for layer l+1
            # right after its last layer-l consumer, hidden under compute)
            wkv = mp.tile([P, 3 * 2 * KH], bf16, name="wkv")
            bks = mp.tile([1, 3 * KH], bf16, name="bks")
            w1s = mp.tile([P, 72 * P], bf16, name="w1s")
            w2s = mp.tile([P, 36 * P], bf16, name="w2s")
            cst = mp.tile([P, 8], f32, name="cst")
            # BN scratch
            t1 = [mp.tile([P, 1], f32, name=f"t1_{i}") for i in range(2)]
            Ac = [mp.tile([P, 1], f32, name=f"Ac{i}") for i in range(2)]
            Bc = [mp.tile([P, 1], f32, name=f"Bc{i}") for i in range(2)]
            sq = [mp.tile([P, 1], f32, name=f"sq{i}") for i in range(2)]
            vart = [mp.tile([P, 1], f32, name=f"vart{i}") for i in range(2)]
            stdt = [mp.tile([P, 1], f32, name=f"stdt{i}") for i in range(2)]

            def copy_on(e, dst, src):
                if e is nc.scalar:
                    nc.scalar.copy(dst, src)
                else:
                    e.tensor_copy(dst, src)

            def load_kqv_weights(l):
                nc.sync.dma_start(wkv[:], wkqvd[l])
                nc.sync.dma_start(bks[:], bkd[l])

            # ---- init ----
            for i in range(2):
                nc.sync.dma_start(x[i][:], xin[i * P:(i + 1) * P, :])
                nc.scalar.copy(xb[i][:], x[i][:])
            nc.sync.dma_start(ident[:], identd)
            for i in range(4):
                nc.vector.memset(opad[i][:], 0)
            for i in range(2):
                nc.vector.memset(h1p[i][:], 0)
            nc.vector.memset(ones1[:], 1.0)
            load_kqv_weights(0)
            nc.sync.dma_start(w1s[:], w1d[0])
            nc.sync.dma_start(w2s[:], w2d[0])
            nc.sync.dma_start(cst[:], cstd[0])

            S3g = S[:].rearrange("p (t g) -> p g t", g=64)
            S3t = S[:].rearrange("p (t g) -> p t g", g=64)
            ab3g = attn[:].rearrange("p (t g) -> p g t", g=64)
            ab3t = attn[:].rearrange("p (t g) -> p t g", g=64)
            abb3t = attnb[:].rearrange("p (t g) -> p t g", g=64)

            for l in range(layers):
                R, T = l + 1, l + 2
                bng = [cst[:, 4 * co + 0:4 * co + 1] for co in range(2)]
                bnb = [cst[:, 4 * co + 1:4 * co + 2] for co in range(2)]
                gob2 = [cst[:, 4 * co + 2:4 * co + 3] for co in range(2)]
                gam = [cst[:, 4 * co + 3:4 * co + 4] for co in range(2)]

                # ---- KQV (both halves; PE streams while copies drain) ----
                cnt = 0
                for hb in range(2):
                    for c, dest in ((0, kbt[l]), (1, vbt[l]), (2, qbt)):
                        for pbh in range(4):
                            pb = hb * 4 + pbh
                            ps = psp.tile([P, KH], f32, name="ps", tag="ps")
                            nc.sync.dma_start(ps[:], bkfd[l, :, c * KH:(c + 1) * KH])
                            nc.tensor.matmul(ps[:], xb[0][:, pb * P:(pb + 1) * P],
                                             wkv[:, (2 * c + 0) * KH:(2 * c + 1) * KH],
                                             start=False, stop=False,
                                             skip_group_check=True)
                            nc.tensor.matmul(ps[:], xb[1][:, pb * P:(pb + 1) * P],
                                             wkv[:, (2 * c + 1) * KH:(2 * c + 2) * KH],
                                             start=False, stop=True,
                                             skip_group_check=True)
                            e = (nc.scalar, nc.vector)[cnt % 2] \
                                if hb == 0 else nc.scalar
                            copy_on(e, dest[:, pb * KH:(pb + 1) * KH], ps[:])
                            cnt += 1
                if l + 1 < layers:
                    load_kqv_weights(l + 1)

                def conv1_part(parts):
                    for co, ck in parts:
                        i0, nr = CHUNKS[ck]
                        nw = PW * nr
                        ps = psp.tile([P, 512], f32, name="ps", tag="ps")
                        for tap in range(9):
                            ty, tx = divmod(tap, 3)
                            base = PW * (i0 + ty) + tx
                            for ci in range(4):
                                off = ((co * 9 + tap) * 4 + ci) * P
                                nc.tensor.matmul(ps[:, 0:nw], w1s[:, off:off + P],
                                                 opad[ci][:, base:base + nw],
                                                 start=(tap == 0 and ci == 0),
                                                 stop=(tap == 8 and ci == 3))
                        copy_on(nc.vector if ck % 2 else nc.scalar,
                                y1[co][:, W * i0:W * (i0 + nr)].rearrange(
                                    "c (i j) -> c i j", j=W),
                                ps[:, 0:nw].rearrange("c (i j) -> c i j", j=PW)[:, :, 0:W])

                # ---- attention halves, pipelined with transpose + conv1 ----
                for hb in range(2):
                    CL = slice(hb * 2048, (hb + 1) * 2048)
                    GS = slice(hb * 32, hb * 32 + 32)
                    # scores
                    for t in range(R):
                        pr = prodp.tile([P, 2048], bf16, name="prod", tag="pr")
                        nc.vector.tensor_mul(pr[:], qbt[:, CL], kbt[t][:, CL])
                        st1 = prodp.tile([P, 2048], bf16, name="st1", tag="pr")
                        pr3 = pr[:].rearrange("p (g d) -> p g d", d=KD)
                        nc.vector.tensor_tensor(
                            st1[:, 0:1024].rearrange("p (g d) -> p g d", d=32),
                            pr3[:, :, 0:32], pr3[:, :, 32:64], OP.add)
                        nc.vector.tensor_reduce(
                            out=S3t[:, t, GS],
                            in_=st1[:, 0:1024].rearrange("p (g d) -> p g d", d=32),
                            axis=AX.X, op=OP.add)
                    nc.vector.memset(S3t[:, R, GS], 0)  # zero-key slot
                    # softmax over T slots
                    nc.vector.tensor_reduce(out=mx[:, GS], in_=S3g[:, GS, 0:T],
                                            axis=AX.X, op=OP.max)
                    nc.vector.tensor_tensor(
                        ab3g[:, GS, 0:T], S3g[:, GS, 0:T],
                        mx[:, GS].unsqueeze(2).broadcast_to([P, 32, T]), OP.subtract)
                    nc.scalar.activation(ab3t[:, 0:T, GS], ab3t[:, 0:T, GS], ACTF.Exp)
                    nc.vector.tensor_reduce(out=zs[:, GS], in_=ab3g[:, GS, 0:T],
                                            axis=AX.X, op=OP.add)
                    nc.vector.reciprocal(zs[:, GS], zs[:, GS])
                    nc.vector.tensor_tensor(
                        ab3g[:, GS, 0:T], ab3g[:, GS, 0:T],
                        zs[:, GS].unsqueeze(2).broadcast_to([P, 32, T]), OP.mult)
                    # sparse top-k (T=5 only): delta = 2nd-smallest = 4th-largest
                    if T > TOPK:
                        first = True
                        for i in range(T):
                            for j in range(i + 1, T):
                                dst = dmin if first else mxp
                                nc.vector.tensor_tensor(dst[:, GS], ab3t[:, i, GS],
                                                        ab3t[:, j, GS], OP.max)
                                if not first:
                                    nc.vector.tensor_tensor(dmin[:, GS], dmin[:, GS],
                                                            mxp[:, GS], OP.min)
                                first = False
                        nc.vector.tensor_scalar_add(dmin[:, GS], dmin[:, GS], EPS)
                        nc.vector.tensor_tensor(
                            ab3g[:, GS, 0:T], ab3g[:, GS, 0:T],
                            dmin[:, GS].unsqueeze(2).broadcast_to([P, 32, T]),
                            OP.subtract)
                        nc.vector.tensor_scalar_max(ab3g[:, GS, 0:T],
                                                    ab3g[:, GS, 0:T], 0.0)
                        nc.vector.tensor_reduce(out=zs[:, GS], in_=ab3g[:, GS, 0:T],
                                                axis=AX.X, op=OP.add)
                        nc.vector.tensor_scalar_add(zs[:, GS], zs[:, GS], EPS)
                        nc.vector.reciprocal(zs[:, GS], zs[:, GS])
                        nc.vector.tensor_tensor(
                            ab3g[:, GS, 0:T], ab3g[:, GS, 0:T],
                            zs[:, GS].unsqueeze(2).broadcast_to([P, 32, T]), OP.mult)
                    nc.vector.tensor_copy(abb3t[:, 0:T, GS], ab3t[:, 0:T, GS])
                    # weighted sum -> obf half. v/obf are d-major
                    # (col = pb*512 + d*8 + h) so the attn broadcast is
                    # packed in the last dim -> DVE 2x mode.
                    for t in range(R):
                        abb = abb3t[:, t, GS].rearrange(
                            "p (b h) -> p b h", h=8).unsqueeze(2).broadcast_to(
                            [P, 4, KD, 8])
                        vv = vbt[t][:, CL].rearrange(
                            "p (b d h) -> p b d h", d=KD, h=8)
                        if t == 0:
                            nc.vector.tensor_tensor(
                                obf[:, CL].rearrange("p (b d h) -> p b d h",
                                                     d=KD, h=8),
                                vv, abb, OP.mult)
                        else:
                            tm = prodp.tile([P, 2048], bf16, name="wtm", tag="pr")
                            nc.vector.tensor_tensor(
                                tm[:].rearrange("p (b d h) -> p b d h", d=KD, h=8),
                                vv, abb, OP.mult)
                            nc.vector.tensor_add(obf[:, CL], obf[:, CL], tm[:])
                    # PE transpose of this half into opad
                    for q in range(4):
                        tp = tpps.tile([P, 512], bf16, name="tp")
                        for pbh in range(4):
                            pb = hb * 4 + pbh
                            nc.tensor.matmul(
                                tp[:, pbh * P:(pbh + 1) * P],
                                obf[:, pb * KH + q * P: pb * KH + (q + 1) * P],
                                ident[:], is_transpose=True, skip_group_check=True)
                        opv = opad[q][:, 0:PHW].rearrange("c (i j) -> c i j", j=PW)
                        copy_on(nc.scalar,
                                opv[:, 1 + 16 * hb:17 + 16 * hb, 1:W + 1],
                                tp[:].rearrange("c (i j) -> c i j", j=W))
                    # conv1: chunk0 of both co overlap attention of half 1
                    if hb == 0:
                        conv1_part([(0, 0), (1, 0)])

                conv1_part([(0, 1), (0, 2)])

                # ---- stats + AllGather per channel-half, pipelined ----
                def stats(i):
                    nc.vector.tensor_reduce(out=st[:, 2 * i:2 * i + 1], in_=y1[i][:],
                                            axis=AX.X, op=OP.add)
                    nc.scalar.square(sqs[i][:], y1[i][:])
                    nc.vector.tensor_reduce(out=st[:, 2 * i + 1:2 * i + 2],
                                            in_=sqs[i][:], axis=AX.X, op=OP.add)
                    cci = dramp.tile([1, 2 * P], f32, name="cci")
                    cco = dramp.tile([ncores, 2 * P], f32, name="cco",
                                     addr_space="Shared")
                    nc.sync.dma_start(cci[0].rearrange("(p j) -> p j", j=2),
                                      st[:, 2 * i:2 * i + 2])
                    nc.gpsimd.collective_compute(
                        "AllGather", OP.bypass,
                        replica_groups=[list(range(ncores))],
                        ins=[cci.opt()], outs=[cco.opt()])
                    nc.sync.dma_start(
                        gst[:, i * 16:(i + 1) * 16].rearrange("p (j s) -> p j s",
                                                              s=ncores),
                        cco[:].rearrange("s (p j) -> p j s", j=2))

                stats(0)
                conv1_part([(1, 1), (1, 2)])
                stats(1)
                if l + 1 < layers:
                    nc.sync.dma_start(w1s[:], w1d[l + 1])
                for co in range(2):
                    nc.scalar.add(x[co][:], x[co][:], gob2[co])

                # ---- BN coef + h1 + conv2 (ci-split overlaps AG latency) ----
                NTOT = float(ncores * HW)

                def bn_h1(i):
                    nc.vector.tensor_reduce(
                        out=gsum[:, 2 * i:2 * i + 2],
                        in_=gst[:, i * 16:(i + 1) * 16].rearrange(
                            "p (j s) -> p j s", s=ncores),
                        axis=AX.X, op=OP.add)
                    nc.vector.tensor_scalar_mul(t1[i][:], gsum[:, 2 * i:2 * i + 1],
                                                1.0 / NTOT)
                    nc.vector.tensor_scalar_mul(vart[i][:],
                                                gsum[:, 2 * i + 1:2 * i + 2],
                                                1.0 / NTOT)
                    nc.vector.tensor_mul(sq[i][:], t1[i][:], t1[i][:])
                    nc.vector.tensor_sub(vart[i][:], vart[i][:], sq[i][:])
                    nc.vector.tensor_scalar_add(vart[i][:], vart[i][:], BN_EPS)
                    nc.scalar.activation(stdt[i][:], vart[i][:], ACTF.Sqrt)
                    nc.vector.reciprocal(stdt[i][:], stdt[i][:])
                    nc.vector.tensor_mul(Ac[i][:], bng[i], stdt[i][:])
                    nc.vector.tensor_mul(sq[i][:], t1[i][:], Ac[i][:])
                    nc.vector.tensor_sub(Bc[i][:], bnb[i], sq[i][:])
                    h1v = h1p[i][:, 0:PHW].rearrange("c (i j) -> c i j", j=PW)
                    nc.scalar.activation(
                        h1v[:, 1:H + 1, 1:W + 1],
                        y1[i][:].rearrange("c (i j) -> c i j", j=W),
                        ACTF.Relu, bias=Bc[i][:], scale=Ac[i][:])

                def conv2_taps(ps2, co, ci, start, stop):
                    for ck, (i0, nr) in enumerate(CHUNKS):
                        nw = PW * nr
                        for tap in range(9):
                            ty, tx = divmod(tap, 3)
                            base = PW * (i0 + ty) + tx
                            off = ((co * 9 + tap) * 2 + ci) * P
                            nc.tensor.matmul(
                                ps2[ck][:, 0:nw], w2s[:, off:off + P],
                                h1p[ci][:, base:base + nw],
                                start=(start and tap == 0),
                                stop=(stop and tap == 8))

                def resid(ps2, co):
                    for ck, (i0, nr) in enumerate(CHUNKS):
                        nw = PW * nr
                        xsl = x[co][:, W * i0:W * (i0 + nr)].rearrange(
                            "c (i j) -> c i j", j=W)
                        nc.vector.scalar_tensor_tensor(
                            out=xsl,
                            in0=ps2[ck][:, 0:nw].rearrange(
                                "c (i j) -> c i j", j=PW)[:, :, 0:W],
                            scalar=gam[co], in1=xsl, op0=OP.mult, op1=OP.add)
                    if l < layers - 1:
                        nc.scalar.copy(xb[co][:], x[co][:])
                    else:
                        nc.sync.dma_start(out[co * P:(co + 1) * P, :], x[co][:])

                # ci-split: both co groups' ci=0 taps hide AG(1) latency
                bn_h1(0)
                ps20 = [psp.tile([P, 512], f32, name="ps", tag="ps")
                        for _ in range(3)]
                ps21 = [psp.tile([P, 512], f32, name="ps", tag="ps")
                        for _ in range(3)]
                conv2_taps(ps20, 0, 0, True, False)
                conv2_taps(ps21, 1, 0, True, False)
                bn_h1(1)
                conv2_taps(ps20, 0, 1, False, True)
                resid(ps20, 0)
                conv2_taps(ps21, 1, 1, False, True)
                if l + 1 < layers:
                    nc.sync.dma_start(w2s[:], w2d[l + 1])
                resid(ps21, 1)
                if l + 1 < layers:
                    nc.sync.dma_start(cst[:], cstd[l + 1])

    nc.compile()
    return nc


def _host_prep(inputs):
    bf = ml_dtypes.bfloat16
    kw, kb = inputs["kw"], inputs["kb"]
    qw, qb = inputs["qw"], inputs["qb"]
    vw, vb = inputs["vw"], inputs["vb"]
    ow1, ow2 = inputs["ow1"], inputs["ow2"]
    ob2, gammas = inputs["ob2"], inputs["gammas"]

    def packkqv(w):  # [L, KH, C] -> [L, P, 2, KH]
        return w.reshape(L, KH, 2, P).transpose(0, 3, 2, 1)

    # v (and o) use d-major channel order: col d*8+h holds row h*64+d.
    # This makes the attention weighted-sum broadcast packed for DVE 2x.
    permdh = np.array([(c % 8) * 64 + c // 8 for c in range(KH)])
    d = {}
    wk3 = np.stack([packkqv(kw), packkqv(vw)[..., permdh],
                    packkqv(qw / 8.0)], axis=2)
    d["wkqvd"] = np.ascontiguousarray(wk3.reshape(L, P, 3 * 2 * KH)).astype(bf)
    bk3 = np.stack([kb, vb[:, permdh], qb / 8.0], axis=1)
    d["bkd"] = np.ascontiguousarray(bk3.reshape(L, 1, 3 * KH)).astype(bf)
    bkf = bk3.reshape(L, 1, 3 * KH).astype(np.float32)
    d["bkfd"] = np.ascontiguousarray(
        np.broadcast_to(bkf, (L, P, 3 * KH))).astype(np.float32)
    # conv1 input channels arrive via the PE transpose of d-major o:
    # opad[k] partition j holds channel c = (j%8)*64 + 16k + j//8.
    cinidx = np.array([[(j % 8) * 64 + 16 * k + j // 8 for j in range(P)]
                       for k in range(4)])
    a1 = ow1[:, :, cinidx]                      # [L, 256, 4, 128, 3, 3]
    a1 = a1.reshape(L, 2, P, 4, P, 3, 3).transpose(0, 4, 1, 5, 6, 3, 2)
    d["w1d"] = np.ascontiguousarray(a1.reshape(L, P, 72 * P)).astype(bf)
    a2 = ow2.reshape(L, 2, P, 2, P, 3, 3).transpose(0, 4, 1, 5, 6, 3, 2)
    d["w2d"] = np.ascontiguousarray(a2.reshape(L, P, 36 * P)).astype(bf)
    cstv = np.zeros((L, 2, P, 4), np.float32)
    cstv[..., 0] = inputs["bn_g"].reshape(L, 2, P)
    cstv[..., 1] = inputs["bn_b"].reshape(L, 2, P)
    cstv[..., 2] = (gammas[:, None] * ob2).reshape(L, 2, P)
    cstv[..., 3] = gammas[:, None, None]
    d["cstd"] = np.ascontiguousarray(
        cstv.transpose(0, 2, 1, 3).reshape(L, P, 8)).astype(np.float32)
    d["identd"] = np.eye(P, dtype=np.float32).astype(bf)
    return d


def kernel(**inputs):
    if "nc" not in _compiled:
        _compiled["nc"] = _build()
    nc = _compiled["nc"]
    shared = _host_prep(inputs)
    x = np.ascontiguousarray(inputs["x"].reshape(B, C, HW)).astype(np.float32)
    in_maps = []
    for c in range(NC):
        m = dict(shared)
        m["xin"] = x[c]
        in_maps.append(m)
    res = bass_utils.run_bass_kernel_spmd(nc, in_maps, core_ids=list(range(NC)))
    outs = np.stack([res.results[c]["out"] for c in range(NC)])
    return outs.reshape(B, C, H, W).astype(np.float32)
